# revision 3
# baseline (speedup 1.0000x reference)
"""Trainium2 Bass kernel for nn_FC_CharNet (branch MLPs + LSTM + head).

Design notes:
- Pure data parallel over batch B=32 -> 4 samples per core on 8 cores.
- All weights/activations kept in a transposed, feature-on-partitions
  layout so every matmul has its contraction dim on partitions and no
  on-device transposes are needed. Host pre-transposes/casts everything.
- Matmuls run in bf16 (fp32 PSUM accumulation); LSTM cell state and all
  elementwise math stay in fp32.
- XW = x @ W_ih.T + b is precomputed for all 64 steps in large-N matmuls
  and streamed through a DRAM scratch buffer; the sequential LSTM loop
  then only does the hx @ W_hh.T matmuls (64 x [128,128]x[128,40]).
"""

import sys

sys.path.insert(0, "/opt/trn_rl_repo")

import numpy as np
import ml_dtypes

import concourse.bass as bass
import concourse.mybir as mybir
import concourse.tile as tile
from concourse.bass_utils import run_bass_kernel_spmd

BF16 = ml_dtypes.bfloat16
F32 = mybir.dt.float32
BF = mybir.dt.bfloat16
AF = mybir.ActivationFunctionType

B, P, S = 32, 10, 64
FEAT = 81 * 35          # 2835
OF = MF = 40
H = 512
C = 256
A = 7
NCORES = 8
BC = B // NCORES        # 4 batch samples per core
ROWS = P * S * BC       # 2560 branch rows per core
LROWS = P * BC          # 40 LSTM rows per core
BLK = 512
NBLK = ROWS // BLK      # 5

# feature-dim chunks of 128 for the board branch (2835 = 22*128 + 19)
KF = [(i * 128, min(128, FEAT - i * 128)) for i in range((FEAT + 127) // 128)]
NKF = len(KF)           # 23


def _split_multi_waits(nc):
    """This container's walrus encodes at most one sync-wait per
    instruction; hoist extra waits onto standalone EventSemaphore
    instructions on the same engine immediately before."""
    n_split = 0
    for bb in nc.main_func.blocks:
        instrs = list(bb.instructions)
        out = []
        changed = False
        for ins in instrs:
            si = getattr(ins, "sync_info", None)
            if si is not None and si.on_wait is not None and len(si.on_wait) > 1:
                waits = list(si.on_wait)
                for i, w in enumerate(waits[:-1]):
                    ev = mybir.InstEventSemaphore(
                        name=f"{ins.name}-sw{i}",
                        engine=ins.engine,
                        ins=[],
                        outs=[],
                        sync_info=mybir.SyncInfo(on_wait=[w], on_update=[]),
                        debug=ins.debug,
                    )
                    nc.register_instruction(ev, overwrite=True)
                    out.append(ev)
                si.on_wait = [waits[-1]]
                n_split += 1
                changed = True
            out.append(ins)
        if changed:
            bb.instructions[:] = out
    return n_split


def _build():
    nc = bass.Bass("TRN2")

    # ---- external inputs (per core) ----
    boardT = nc.dram_tensor("boardT", [FEAT, ROWS], BF, kind="ExternalInput")
    orderT = nc.dram_tensor("orderT", [OF, ROWS], BF, kind="ExternalInput")
    msgT = nc.dram_tensor("msgT", [MF, ROWS], BF, kind="ExternalInput")
    wb1t = nc.dram_tensor("wb1t", [FEAT, 1024], BF, kind="ExternalInput")
    wb2t = nc.dram_tensor("wb2t", [1024, 256], BF, kind="ExternalInput")
    wb3t = nc.dram_tensor("wb3t", [256, 256], BF, kind="ExternalInput")
    wo1t = nc.dram_tensor("wo1t", [OF, 128], BF, kind="ExternalInput")
    wo2t = nc.dram_tensor("wo2t", [128, 128], BF, kind="ExternalInput")
    wm1t = nc.dram_tensor("wm1t", [MF, 64], BF, kind="ExternalInput")
    wm2t = nc.dram_tensor("wm2t", [64, 128], BF, kind="ExternalInput")
    wiht = nc.dram_tensor("wiht", [H, 4 * H], BF, kind="ExternalInput")
    whht = nc.dram_tensor("whht", [H, 4 * H], BF, kind="ExternalInput")
    w1t = nc.dram_tensor("w1t", [H + 2 * A, C], BF, kind="ExternalInput")
    w2t = nc.dram_tensor("w2t", [C, C], BF, kind="ExternalInput")
    # biases, feature-on-partition layout [128, nchunks]
    bb1 = nc.dram_tensor("bb1", [128, 8], F32, kind="ExternalInput")
    bb2 = nc.dram_tensor("bb2", [128, 2], F32, kind="ExternalInput")
    bb3 = nc.dram_tensor("bb3", [128, 2], F32, kind="ExternalInput")
    bo1 = nc.dram_tensor("bo1", [128, 1], F32, kind="ExternalInput")
    bo2 = nc.dram_tensor("bo2", [128, 1], F32, kind="ExternalInput")
    bm1 = nc.dram_tensor("bm1", [64, 1], F32, kind="ExternalInput")
    bm2 = nc.dram_tensor("bm2", [128, 1], F32, kind="ExternalInput")
    bg = nc.dram_tensor("bg", [128, 16], F32, kind="ExternalInput")
    b1 = nc.dram_tensor("b1", [128, 2], F32, kind="ExternalInput")
    b2 = nc.dram_tensor("b2", [128, 2], F32, kind="ExternalInput")
    h0 = nc.dram_tensor("h0", [H, LROWS], BF, kind="ExternalInput")
    c0 = nc.dram_tensor("c0", [H, LROWS], F32, kind="ExternalInput")
    omt = nc.dram_tensor("omt", [2 * A, LROWS], BF, kind="ExternalInput")
    echar = nc.dram_tensor("echar", [C, LROWS], F32, kind="ExternalOutput")

    with tile.TileContext(nc) as tc:
        with (
            tc.tile_pool(name="wpool", bufs=1) as wpool,
            tc.tile_pool(name="dpool", bufs=1, space="DRAM") as dpool,
            tc.tile_pool(name="bpool", bufs=2) as bpool,
            tc.tile_pool(name="apool", bufs=2) as apool,
            tc.tile_pool(name="xwpool", bufs=4) as xwpool,
            tc.tile_pool(name="state", bufs=1) as state,
            tc.tile_pool(name="lpool", bufs=2) as lpool,
        ):
            # ---- load weights to SBUF ----
            wb1s = wpool.tile([128, NKF, 1024], BF, tag="wb1s")
            for k, (k0, kn) in enumerate(KF):
                nc.sync.dma_start(out=wb1s[:kn, k, :], in_=wb1t[k0:k0 + kn, :])
            wb2s = wpool.tile([128, 8, 256], BF, tag="wb2s")
            for k in range(8):
                nc.sync.dma_start(out=wb2s[:, k, :], in_=wb2t[k * 128:(k + 1) * 128, :])
            wb3s = wpool.tile([128, 2, 256], BF, tag="wb3s")
            for k in range(2):
                nc.sync.dma_start(out=wb3s[:, k, :], in_=wb3t[k * 128:(k + 1) * 128, :])
            wo1s = wpool.tile([OF, 128], BF, tag="wo1s")
            nc.sync.dma_start(out=wo1s[:], in_=wo1t[:])
            wo2s = wpool.tile([128, 128], BF, tag="wo2s")
            nc.sync.dma_start(out=wo2s[:], in_=wo2t[:])
            wm1s = wpool.tile([MF, 64], BF, tag="wm1s")
            nc.sync.dma_start(out=wm1s[:], in_=wm1t[:])
            wm2s = wpool.tile([64, 128], BF, tag="wm2s")
            nc.sync.dma_start(out=wm2s[:], in_=wm2t[:])
            wihs = wpool.tile([128, 4, 4 * H], BF, tag="wihs")
            for k in range(4):
                nc.sync.dma_start(out=wihs[:, k, :], in_=wiht[k * 128:(k + 1) * 128, :])
            whhs = wpool.tile([128, 4, 4 * H], BF, tag="whhs")
            for k in range(4):
                nc.sync.dma_start(out=whhs[:, k, :], in_=whht[k * 128:(k + 1) * 128, :])
            w1s = wpool.tile([128, 5, C], BF, tag="w1s")
            for k in range(4):
                nc.sync.dma_start(out=w1s[:, k, :], in_=w1t[k * 128:(k + 1) * 128, :])
            nc.sync.dma_start(out=w1s[:2 * A, 4, :], in_=w1t[H:H + 2 * A, :])
            w2s = wpool.tile([128, 2, C], BF, tag="w2s")
            for k in range(2):
                nc.sync.dma_start(out=w2s[:, k, :], in_=w2t[k * 128:(k + 1) * 128, :])

            bb1s = wpool.tile([128, 8], F32, tag="bb1s")
            nc.sync.dma_start(out=bb1s[:], in_=bb1[:])
            bb2s = wpool.tile([128, 2], F32, tag="bb2s")
            nc.sync.dma_start(out=bb2s[:], in_=bb2[:])
            bb3s = wpool.tile([128, 2], F32, tag="bb3s")
            nc.sync.dma_start(out=bb3s[:], in_=bb3[:])
            bo1s = wpool.tile([128, 1], F32, tag="bo1s")
            nc.sync.dma_start(out=bo1s[:], in_=bo1[:])
            bo2s = wpool.tile([128, 1], F32, tag="bo2s")
            nc.sync.dma_start(out=bo2s[:], in_=bo2[:])
            bm1s = wpool.tile([64, 1], F32, tag="bm1s")
            nc.sync.dma_start(out=bm1s[:], in_=bm1[:])
            bm2s = wpool.tile([128, 1], F32, tag="bm2s")
            nc.sync.dma_start(out=bm2s[:], in_=bm2[:])
            bgs = wpool.tile([128, 16], F32, tag="bgs")
            nc.sync.dma_start(out=bgs[:], in_=bg[:])
            b1s = wpool.tile([128, 2], F32, tag="b1s")
            nc.sync.dma_start(out=b1s[:], in_=b1[:])
            b2s = wpool.tile([128, 2], F32, tag="b2s")
            nc.sync.dma_start(out=b2s[:], in_=b2[:])
            oms = wpool.tile([2 * A, LROWS], BF, tag="oms")
            nc.sync.dma_start(out=oms[:], in_=omt[:])

            # DRAM scratch for precomputed xW (fp32, gate-chunk-major)
            xwt = dpool.tile([128, 16, ROWS], F32, tag="xwt")

            # ================= branch MLPs + XW precompute =================
            with tc.tile_pool(name="psA", bufs=4, space="PSUM") as psA:
                for r in range(NBLK):
                    cols = slice(r * BLK, (r + 1) * BLK)
                    bt = bpool.tile([128, NKF, BLK], BF, tag="bt")
                    for k, (k0, kn) in enumerate(KF):
                        nc.sync.dma_start(out=bt[:kn, k, :], in_=boardT[k0:k0 + kn, cols])
                    ot = bpool.tile([OF, BLK], BF, tag="ot")
                    nc.sync.dma_start(out=ot[:], in_=orderT[:, cols])
                    mt = bpool.tile([MF, BLK], BF, tag="mt")
                    nc.sync.dma_start(out=mt[:], in_=msgT[:, cols])

                    # board L1: [2835 -> 1024]
                    act1 = apool.tile([128, 8, BLK], BF, tag="act1")
                    for m in range(8):
                        ps = psA.tile([128, BLK], F32, tag="ps")
                        for k, (k0, kn) in enumerate(KF):
                            nc.tensor.matmul(
                                ps[:], wb1s[:kn, k, m * 128:(m + 1) * 128],
                                bt[:kn, k, :], start=(k == 0), stop=(k == NKF - 1))
                        nc.scalar.activation(act1[:, m, :], ps[:], AF.Relu,
                                             bias=bb1s[:, m:m + 1])
                    # board L2: [1024 -> 256]
                    act2 = apool.tile([128, 2, BLK], BF, tag="act2")
                    for m in range(2):
                        ps = psA.tile([128, BLK], F32, tag="ps")
                        for k in range(8):
                            nc.tensor.matmul(
                                ps[:], wb2s[:, k, m * 128:(m + 1) * 128],
                                act1[:, k, :], start=(k == 0), stop=(k == 7))
                        nc.scalar.activation(act2[:, m, :], ps[:], AF.Relu,
                                             bias=bb2s[:, m:m + 1])
                    # x = [bfeat(256); ofeat(128); mfeat(128)]  (H on partitions)
                    xt = apool.tile([128, 4, BLK], BF, tag="xt")
                    # board L3: [256 -> 256]
                    for m in range(2):
                        ps = psA.tile([128, BLK], F32, tag="ps")
                        for k in range(2):
                            nc.tensor.matmul(
                                ps[:], wb3s[:, k, m * 128:(m + 1) * 128],
                                act2[:, k, :], start=(k == 0), stop=(k == 1))
                        nc.scalar.activation(xt[:, m, :], ps[:], AF.Relu,
                                             bias=bb3s[:, m:m + 1])
                    # order branch: [40 -> 128 -> 128]
                    ps = psA.tile([128, BLK], F32, tag="ps")
                    nc.tensor.matmul(ps[:], wo1s[:], ot[:], start=True, stop=True)
                    o1 = apool.tile([128, BLK], BF, tag="o1")
                    nc.scalar.activation(o1[:], ps[:], AF.Relu, bias=bo1s[:, 0:1])
                    ps = psA.tile([128, BLK], F32, tag="ps")
                    nc.tensor.matmul(ps[:], wo2s[:], o1[:], start=True, stop=True)
                    nc.scalar.activation(xt[:, 2, :], ps[:], AF.Relu, bias=bo2s[:, 0:1])
                    # message branch: [40 -> 64 -> 128]
                    ps = psA.tile([128, BLK], F32, tag="ps")
                    nc.tensor.matmul(ps[:64, :], wm1s[:], mt[:], start=True, stop=True)
                    m1 = apool.tile([64, BLK], BF, tag="m1")
                    nc.scalar.activation(m1[:], ps[:64, :], AF.Relu, bias=bm1s[:, 0:1])
                    ps = psA.tile([128, BLK], F32, tag="ps")
                    nc.tensor.matmul(ps[:], wm2s[:], m1[:], start=True, stop=True)
                    nc.scalar.activation(xt[:, 3, :], ps[:], AF.Relu, bias=bm2s[:, 0:1])
                    # XW precompute: [512 -> 2048], + (b_ih + b_hh)
                    for m in range(16):
                        ps = psA.tile([128, BLK], F32, tag="ps")
                        for k in range(4):
                            nc.tensor.matmul(
                                ps[:], wihs[:, k, m * 128:(m + 1) * 128],
                                xt[:, k, :], start=(k == 0), stop=(k == 3))
                        xw = xwpool.tile([128, BLK], F32, tag="xw")
                        nc.scalar.activation(xw[:], ps[:], AF.Identity,
                                             bias=bgs[:, m:m + 1])
                        nc.sync.dma_start(out=xwt[:, m, cols], in_=xw[:])

            # ========================= LSTM =========================
            hs = [state.tile([128, LROWS], BF, tag=f"h{k}", name=f"h{k}") for k in range(4)]
            cs = [state.tile([128, LROWS], F32, tag=f"c{k}", name=f"c{k}") for k in range(4)]
            qs = [state.tile([128, LROWS], F32, tag=f"q{k}", name=f"q{k}") for k in range(4)]
            for k in range(4):
                nc.sync.dma_start(out=hs[k][:], in_=h0[k * 128:(k + 1) * 128, :])
                nc.sync.dma_start(out=cs[k][:], in_=c0[k * 128:(k + 1) * 128, :])
                nc.vector.memset(qs[k][:], 0.0)

            with tc.tile_pool(name="psB", bufs=8, space="PSUM") as psB:
                for t in range(S):
                    xwin = lpool.tile([128, 16, LROWS], F32, tag="xwin")
                    nc.sync.dma_start(
                        out=xwin[:], in_=xwt[:, :, t * LROWS:(t + 1) * LROWS])
                    ga = [None] * 16
                    # gate chunks: m//4 = gate (i,f,g,o), m%4 = H-chunk
                    for m in range(16):
                        ps = psB.tile([128, LROWS], F32, tag="gps")
                        for k in range(4):
                            nc.tensor.matmul(
                                ps[:], whhs[:, k, m * 128:(m + 1) * 128],
                                hs[k][:], start=(k == 0), stop=(k == 3))
                        pre = lpool.tile([128, LROWS], F32, tag=f"pre{m}")
                        nc.vector.tensor_add(pre[:], ps[:], xwin[:, m, :])
                        g = lpool.tile([128, LROWS], F32, tag=f"ga{m}")
                        fn = AF.Tanh if m // 4 == 2 else AF.Sigmoid
                        nc.scalar.activation(g[:], pre[:], fn)
                        ga[m] = g
                    for k in range(4):
                        ig = lpool.tile([128, LROWS], F32, tag=f"ig{k}")
                        nc.vector.tensor_mul(ig[:], ga[k][:], ga[8 + k][:])
                        fc = lpool.tile([128, LROWS], F32, tag=f"fc{k}")
                        nc.vector.tensor_mul(fc[:], ga[4 + k][:], cs[k][:])
                        nc.vector.tensor_add(cs[k][:], ig[:], fc[:])
                        tcell = lpool.tile([128, LROWS], F32, tag=f"tc{k}")
                        nc.scalar.activation(tcell[:], cs[k][:], AF.Tanh)
                        hf = lpool.tile([128, LROWS], F32, tag=f"hf{k}")
                        nc.vector.tensor_mul(hf[:], ga[12 + k][:], tcell[:])
                        nc.vector.tensor_add(qs[k][:], qs[k][:], hf[:])
                        nc.vector.tensor_copy(hs[k][:], hf[:])

                # ===================== head =====================
                qb = []
                for k in range(4):
                    qk = lpool.tile([128, LROWS], BF, tag=f"qb{k}", name=f"qb{k}")
                    nc.vector.tensor_copy(qk[:], qs[k][:])
                    qb.append(qk)
                e1 = []
                for m in range(2):
                    ps = psB.tile([128, LROWS], F32, tag="gps")
                    for k in range(4):
                        nc.tensor.matmul(
                            ps[:], w1s[:, k, m * 128:(m + 1) * 128],
                            qb[k][:], start=(k == 0), stop=False)
                    nc.tensor.matmul(
                        ps[:], w1s[:2 * A, 4, m * 128:(m + 1) * 128],
                        oms[:], start=False, stop=True)
                    e1m = lpool.tile([128, LROWS], BF, tag=f"e1{m}")
                    nc.scalar.activation(e1m[:], ps[:], AF.Relu, bias=b1s[:, m:m + 1])
                    e1.append(e1m)
                for m in range(2):
                    ps = psB.tile([128, LROWS], F32, tag="gps")
                    for k in range(2):
                        nc.tensor.matmul(
                            ps[:], w2s[:, k, m * 128:(m + 1) * 128],
                            e1[k][:], start=(k == 0), stop=(k == 1))
                    eo = lpool.tile([128, LROWS], F32, tag=f"eo{m}")
                    nc.scalar.activation(eo[:], ps[:], AF.Identity, bias=b2s[:, m:m + 1])
                    nc.sync.dma_start(out=echar[m * 128:(m + 1) * 128, :], in_=eo[:])

    _split_multi_waits(nc)
    return nc


_NC_CACHE = {}


def _get_nc():
    if "nc" not in _NC_CACHE:
        _NC_CACHE["nc"] = _build()
    return _NC_CACHE["nc"]


def _prep(inputs):
    f32 = np.float32
    board = np.asarray(inputs["board"], dtype=f32).reshape(B, P, S, FEAT)
    order = np.asarray(inputs["order"], dtype=f32)
    message = np.asarray(inputs["message"], dtype=f32)
    other = np.asarray(inputs["other_ind"], dtype=f32)
    me = np.asarray(inputs["me_ind"], dtype=f32)
    hx0 = np.asarray(inputs["hx0"], dtype=f32)
    cx0 = np.asarray(inputs["cx0"], dtype=f32)

    w = {}
    w["wb1t"] = np.ascontiguousarray(inputs["Wb1"].T).astype(BF16)
    w["wb2t"] = np.ascontiguousarray(inputs["Wb2"].T).astype(BF16)
    w["wb3t"] = np.ascontiguousarray(inputs["Wb3"].T).astype(BF16)
    w["wo1t"] = np.ascontiguousarray(inputs["Wo1"].T).astype(BF16)
    w["wo2t"] = np.ascontiguousarray(inputs["Wo2"].T).astype(BF16)
    w["wm1t"] = np.ascontiguousarray(inputs["Wm1"].T).astype(BF16)
    w["wm2t"] = np.ascontiguousarray(inputs["Wm2"].T).astype(BF16)
    w["wiht"] = np.ascontiguousarray(inputs["W_ih"].T).astype(BF16)
    w["whht"] = np.ascontiguousarray(inputs["W_hh"].T).astype(BF16)
    w["w1t"] = np.ascontiguousarray(inputs["W1"].T).astype(BF16)
    w["w2t"] = np.ascontiguousarray(inputs["W2"].T).astype(BF16)
    w["bb1"] = np.ascontiguousarray(
        np.asarray(inputs["bb1"], f32).reshape(8, 128).T)
    w["bb2"] = np.ascontiguousarray(
        np.asarray(inputs["bb2"], f32).reshape(2, 128).T)
    w["bb3"] = np.ascontiguousarray(
        np.asarray(inputs["bb3"], f32).reshape(2, 128).T)
    w["bo1"] = np.asarray(inputs["bo1"], f32).reshape(128, 1)
    w["bo2"] = np.asarray(inputs["bo2"], f32).reshape(128, 1)
    w["bm1"] = np.asarray(inputs["bm1"], f32).reshape(64, 1)
    w["bm2"] = np.asarray(inputs["bm2"], f32).reshape(128, 1)
    w["bg"] = np.ascontiguousarray(
        (np.asarray(inputs["b_ih"], f32)
         + np.asarray(inputs["b_hh"], f32)).reshape(16, 128).T)
    w["b1"] = np.ascontiguousarray(
        np.asarray(inputs["b1"], f32).reshape(2, 128).T)
    w["b2"] = np.ascontiguousarray(
        np.asarray(inputs["b2"], f32).reshape(2, 128).T)

    board_bf = board.astype(BF16)        # [B,P,S,FEAT]
    order_bf = order.astype(BF16)
    msg_bf = message.astype(BF16)
    hT = hx0.transpose(2, 0, 1)          # [H,P,B]
    cT = cx0.transpose(2, 0, 1)
    oT = other.transpose(2, 1, 0)        # [A,P,B]
    mT = me.transpose(2, 1, 0)

    in_maps = []
    for c in range(NCORES):
        bsl = slice(c * BC, (c + 1) * BC)
        im = dict(w)
        im["boardT"] = np.ascontiguousarray(
            board_bf[bsl].transpose(3, 2, 1, 0)).reshape(FEAT, ROWS)
        im["orderT"] = np.ascontiguousarray(
            order_bf[bsl].transpose(3, 2, 1, 0)).reshape(OF, ROWS)
        im["msgT"] = np.ascontiguousarray(
            msg_bf[bsl].transpose(3, 2, 1, 0)).reshape(MF, ROWS)
        im["h0"] = np.ascontiguousarray(hT[:, :, bsl]).reshape(H, LROWS).astype(BF16)
        im["c0"] = np.ascontiguousarray(cT[:, :, bsl]).reshape(H, LROWS)
        im["omt"] = np.concatenate(
            [np.ascontiguousarray(oT[:, :, bsl]).reshape(A, LROWS),
             np.ascontiguousarray(mT[:, :, bsl]).reshape(A, LROWS)],
            axis=0).astype(BF16)
        in_maps.append(im)
    return in_maps


_RUN_OPTS = {"trace": False}
_LAST_RES = {}


def kernel(**inputs):
    nc = _get_nc()
    in_maps = _prep(inputs)
    res = run_bass_kernel_spmd(nc, in_maps, list(range(NCORES)),
                               trace=_RUN_OPTS["trace"])
    _LAST_RES["res"] = res
    ect = np.empty((P, B, C), dtype=np.float32)
    for c in range(NCORES):
        out_c = res.results[c]["echar"]              # [C, LROWS]
        ect[:, c * BC:(c + 1) * BC, :] = (
            out_c.reshape(C, P, BC).transpose(1, 2, 0))
    final = ect.sum(axis=0)
    return final, ect


# revision 31
# speedup vs baseline: 1.4290x; 1.4290x over previous
"""Trainium2 Bass kernel for nn_FC_CharNet (branch MLPs + LSTM + head).

Design notes:
- Pure data parallel over batch B=32 -> 4 samples per core on 8 cores.
- All weights/activations kept in a transposed, feature-on-partitions
  layout so every matmul has its contraction dim on partitions and no
  on-device transposes are needed. Host pre-transposes/casts everything.
- Matmuls run in bf16 (fp32 PSUM accumulation); LSTM cell state and all
  elementwise math stay in fp32.
- XW = x @ W_ih.T + b is precomputed for all 64 steps in large-N matmuls
  and streamed through a DRAM scratch buffer; the sequential LSTM loop
  then only does the hx @ W_hh.T matmuls (64 x [128,128]x[128,40]).
"""

import sys

sys.path.insert(0, "/opt/trn_rl_repo")

import numpy as np
import ml_dtypes

import concourse.bass as bass
import concourse.mybir as mybir
import concourse.tile as tile
from concourse.bass_utils import run_bass_kernel_spmd

BF16 = ml_dtypes.bfloat16
F32 = mybir.dt.float32
BF = mybir.dt.bfloat16
AF = mybir.ActivationFunctionType

B, P, S = 32, 10, 64
FEAT = 81 * 35          # 2835
FEATP = 2944            # FEAT padded to a multiple of 128 (23 chunks)
OF = MF = 40
H = 512
C = 256
A = 7
NCORES = 8
BC = B // NCORES        # 4 batch samples per core
ROWS = P * S * BC       # 2560 branch rows per core
LROWS = P * BC          # 40 LSTM rows per core
BLK = 320
NBLK = ROWS // BLK      # 8

NKF = FEATP // 128      # 23


def _split_multi_waits(nc):
    """This container's walrus encodes at most one sync-wait per
    instruction; hoist extra waits onto standalone EventSemaphore
    instructions on the same engine immediately before."""
    n_split = 0
    for bb in nc.main_func.blocks:
        instrs = list(bb.instructions)
        out = []
        changed = False
        for ins in instrs:
            si = getattr(ins, "sync_info", None)
            if si is not None and si.on_wait is not None and len(si.on_wait) > 1:
                waits = list(si.on_wait)
                for i, w in enumerate(waits[:-1]):
                    ev = mybir.InstEventSemaphore(
                        name=f"{ins.name}-sw{i}",
                        engine=ins.engine,
                        ins=[],
                        outs=[],
                        sync_info=mybir.SyncInfo(on_wait=[w], on_update=[]),
                        debug=ins.debug,
                    )
                    nc.register_instruction(ev, overwrite=True)
                    out.append(ev)
                si.on_wait = [waits[-1]]
                n_split += 1
                changed = True
            out.append(ins)
        if changed:
            bb.instructions[:] = out
    return n_split


def _build():
    nc = bass.Bass("TRN2")

    # ---- external inputs (per core) ----
    boardT = nc.dram_tensor("boardT", [FEATP, ROWS], BF, kind="ExternalInput")
    orderT = nc.dram_tensor("orderT", [OF, ROWS], BF, kind="ExternalInput")
    msgT = nc.dram_tensor("msgT", [MF, ROWS], BF, kind="ExternalInput")
    wb1t = nc.dram_tensor("wb1t", [FEATP, 1024], BF, kind="ExternalInput")
    wb2t = nc.dram_tensor("wb2t", [1024, 256], BF, kind="ExternalInput")
    wb3t = nc.dram_tensor("wb3t", [256, 256], BF, kind="ExternalInput")
    wo1t = nc.dram_tensor("wo1t", [OF, 128], BF, kind="ExternalInput")
    wo2t = nc.dram_tensor("wo2t", [128, 128], BF, kind="ExternalInput")
    wm1t = nc.dram_tensor("wm1t", [MF, 64], BF, kind="ExternalInput")
    wm2t = nc.dram_tensor("wm2t", [64, 128], BF, kind="ExternalInput")
    wiht = nc.dram_tensor("wiht", [H, 4 * H], BF, kind="ExternalInput")
    whht = nc.dram_tensor("whht", [H, 4 * H], BF, kind="ExternalInput")
    w1t = nc.dram_tensor("w1t", [H + 2 * A, C], BF, kind="ExternalInput")
    w2t = nc.dram_tensor("w2t", [C, C], BF, kind="ExternalInput")
    # biases, feature-on-partition layout [128, nchunks]
    bb1 = nc.dram_tensor("bb1", [128, 8], F32, kind="ExternalInput")
    bb2 = nc.dram_tensor("bb2", [128, 2], F32, kind="ExternalInput")
    bb3 = nc.dram_tensor("bb3", [128, 2], F32, kind="ExternalInput")
    bo1 = nc.dram_tensor("bo1", [128, 1], F32, kind="ExternalInput")
    bo2 = nc.dram_tensor("bo2", [128, 1], F32, kind="ExternalInput")
    bm1 = nc.dram_tensor("bm1", [64, 1], F32, kind="ExternalInput")
    bm2 = nc.dram_tensor("bm2", [128, 1], F32, kind="ExternalInput")
    bg = nc.dram_tensor("bg", [128, 16], F32, kind="ExternalInput")
    b1 = nc.dram_tensor("b1", [128, 2], F32, kind="ExternalInput")
    b2 = nc.dram_tensor("b2", [128, 2], F32, kind="ExternalInput")
    h0 = nc.dram_tensor("h0", [H, LROWS], BF, kind="ExternalInput")
    c0 = nc.dram_tensor("c0", [H, LROWS], F32, kind="ExternalInput")
    omt = nc.dram_tensor("omt", [2 * A, LROWS], BF, kind="ExternalInput")
    ident = nc.dram_tensor("ident", [128, 128], BF, kind="ExternalInput")
    echar = nc.dram_tensor("echar", [C, LROWS], F32, kind="ExternalOutput")

    with tile.TileContext(nc) as tc:
        with (
            tc.tile_pool(name="wpool", bufs=1) as wpool,
            tc.tile_pool(name="dpool", bufs=1, space="DRAM") as dpool,
            tc.tile_pool(name="bpool", bufs=3) as bpool,
            tc.tile_pool(name="apool", bufs=2) as apool,
            tc.tile_pool(name="xwpool", bufs=3) as xwpool,
            tc.tile_pool(name="state", bufs=1) as state,
            tc.tile_pool(name="lpool", bufs=2) as lpool,
        ):
            # ---- load weights to SBUF ----
            wb1s = wpool.tile([128, NKF, 1024], BF, tag="wb1s")
            for k, (k0, kn) in enumerate(KF):
                nc.sync.dma_start(out=wb1s[:kn, k, :], in_=wb1t[k0:k0 + kn, :])
            wb2s = wpool.tile([128, 8, 256], BF, tag="wb2s")
            for k in range(8):
                nc.sync.dma_start(out=wb2s[:, k, :], in_=wb2t[k * 128:(k + 1) * 128, :])
            wb3s = wpool.tile([128, 2, 256], BF, tag="wb3s")
            for k in range(2):
                nc.sync.dma_start(out=wb3s[:, k, :], in_=wb3t[k * 128:(k + 1) * 128, :])
            wo1s = wpool.tile([OF, 128], BF, tag="wo1s")
            nc.sync.dma_start(out=wo1s[:], in_=wo1t[:])
            wo2s = wpool.tile([128, 128], BF, tag="wo2s")
            nc.sync.dma_start(out=wo2s[:], in_=wo2t[:])
            wm1s = wpool.tile([MF, 64], BF, tag="wm1s")
            nc.sync.dma_start(out=wm1s[:], in_=wm1t[:])
            wm2s = wpool.tile([64, 128], BF, tag="wm2s")
            nc.sync.dma_start(out=wm2s[:], in_=wm2t[:])
            wihs = wpool.tile([128, 4, 4 * H], BF, tag="wihs")
            for k in range(4):
                nc.sync.dma_start(out=wihs[:, k, :], in_=wiht[k * 128:(k + 1) * 128, :])
            whhs = wpool.tile([128, 4, 4 * H], BF, tag="whhs")
            for k in range(4):
                nc.sync.dma_start(out=whhs[:, k, :], in_=whht[k * 128:(k + 1) * 128, :])
            w1s = wpool.tile([128, 5, C], BF, tag="w1s")
            for k in range(4):
                nc.sync.dma_start(out=w1s[:, k, :], in_=w1t[k * 128:(k + 1) * 128, :])
            nc.sync.dma_start(out=w1s[:2 * A, 4, :], in_=w1t[H:H + 2 * A, :])
            w2s = wpool.tile([128, 2, C], BF, tag="w2s")
            for k in range(2):
                nc.sync.dma_start(out=w2s[:, k, :], in_=w2t[k * 128:(k + 1) * 128, :])

            bb1s = wpool.tile([128, 8], F32, tag="bb1s")
            nc.sync.dma_start(out=bb1s[:], in_=bb1[:])
            bb2s = wpool.tile([128, 2], F32, tag="bb2s")
            nc.sync.dma_start(out=bb2s[:], in_=bb2[:])
            bb3s = wpool.tile([128, 2], F32, tag="bb3s")
            nc.sync.dma_start(out=bb3s[:], in_=bb3[:])
            bo1s = wpool.tile([128, 1], F32, tag="bo1s")
            nc.sync.dma_start(out=bo1s[:], in_=bo1[:])
            bo2s = wpool.tile([128, 1], F32, tag="bo2s")
            nc.sync.dma_start(out=bo2s[:], in_=bo2[:])
            bm1s = wpool.tile([64, 1], F32, tag="bm1s")
            nc.sync.dma_start(out=bm1s[:], in_=bm1[:])
            bm2s = wpool.tile([128, 1], F32, tag="bm2s")
            nc.sync.dma_start(out=bm2s[:], in_=bm2[:])
            bgs = wpool.tile([128, 16], F32, tag="bgs")
            nc.sync.dma_start(out=bgs[:], in_=bg[:])
            b1s = wpool.tile([128, 2], F32, tag="b1s")
            nc.sync.dma_start(out=b1s[:], in_=b1[:])
            b2s = wpool.tile([128, 2], F32, tag="b2s")
            nc.sync.dma_start(out=b2s[:], in_=b2[:])
            oms = wpool.tile([2 * A, LROWS], BF, tag="oms")
            nc.sync.dma_start(out=oms[:], in_=omt[:])
            ids = wpool.tile([128, 128], BF, tag="ids")
            nc.sync.dma_start(out=ids[:], in_=ident[:])

            # DRAM scratch for precomputed xW (fp32, gate-chunk-major)
            xwt = dpool.tile([128, 16, ROWS], BF, tag="xwt")

            # ===== branch MLPs + XW precompute, interleaved with the LSTM =====
            # The LSTM recurrence is latency-bound (~1.5us/step of PE idle
            # while the serial gate->cell chain runs), so branch-matmul work
            # for later blocks is emitted in small pieces BETWEEN the LSTM
            # steps: the in-order PE queue then fills the recurrence gaps
            # with branch matmuls. Blocks are 320 rows = exactly 8 LSTM
            # steps of xW, and the LSTM region r only needs block r's xW.
            hh = [state.tile([128, LROWS], BF, tag=f"hh{k}", name=f"hh{k}")
                  for k in range(4)]
            cc = [state.tile([128, LROWS], F32, tag=f"cc{k}", name=f"cc{k}")
                  for k in range(4)]
            qq = [state.tile([128, LROWS], F32, tag=f"qq{k}", name=f"qq{k}")
                  for k in range(4)]
            for k in range(4):
                nc.sync.dma_start(out=hh[k][:], in_=h0[k * 128:(k + 1) * 128, :])
                nc.sync.dma_start(out=cc[k][:], in_=c0[k * 128:(k + 1) * 128, :])
                nc.vector.memset(qq[k][:], 0.0)

            with (
                tc.tile_pool(name="wb1pool", bufs=1) as wb1pool,
                tc.tile_pool(name="bpool", bufs=3) as bpool,
                tc.tile_pool(name="apool", bufs=2) as apool,
                tc.tile_pool(name="xwpool", bufs=3) as xwpool,
                tc.tile_pool(name="lpool", bufs=2) as lpool,
                tc.tile_pool(name="psA", bufs=3, space="PSUM") as psA,
                tc.tile_pool(name="psB", bufs=4, space="PSUM") as psB,
            ):
                KG = [(0, 6), (6, 6), (12, 6), (18, 5)]  # k-groups of chunks
                wb1s_g = []
                for gi, (g0, gn) in enumerate(KG):
                    wt = wb1pool.tile([128, gn, 1024], BF, tag=f"wb1s{gi}",
                                      name=f"wb1s{gi}")
                    nc.gpsimd.dma_start(
                        out=wt[:],
                        in_=wb1t[g0 * 128:(g0 + gn) * 128, :]
                        .rearrange("(k p) c -> p k c", p=128))
                    wb1s_g.append(wt)

                # ---- branch work, as a FIFO of small emit-thunks ----
                blocks = {}

                def th_dma(r):
                    def f():
                        cols = slice(r * BLK, (r + 1) * BLK)
                        d = {}
                        d["bt"] = []
                        for gi, (g0, gn) in enumerate(KG):
                            btt = bpool.tile([128, gn, BLK], BF, tag=f"bt{gi}",
                                             name=f"bt{gi}_{r}")
                            nc.gpsimd.dma_start(
                                out=btt[:],
                                in_=boardT[g0 * 128:(g0 + gn) * 128, cols]
                                .rearrange("(k p) c -> p k c", p=128))
                            d["bt"].append(btt)
                        d["ot"] = bpool.tile([OF, BLK], BF, tag="ot",
                                             name=f"ot{r}")
                        nc.sync.dma_start(out=d["ot"][:], in_=orderT[:, cols])
                        d["mt"] = bpool.tile([MF, BLK], BF, tag="mt",
                                             name=f"mt{r}")
                        nc.sync.dma_start(out=d["mt"][:], in_=msgT[:, cols])
                        d["act1"] = apool.tile([128, 8, BLK], BF, tag="act1",
                                               name=f"act1_{r}")
                        d["act2"] = apool.tile([128, 2, BLK], BF, tag="act2",
                                               name=f"act2_{r}")
                        d["xt"] = apool.tile([128, 4, BLK], BF, tag="xt",
                                             name=f"xt{r}")

                        blocks[r] = d
                    return f

                def th_l1(r, m):
                    def f():
                        d = blocks[r]
                        ps = psA.tile([128, BLK], F32, tag="ps", name=f"psl1_{r}_{m}")
                        for k in range(NKF):
                            gi, ki = (k // 6, k % 6)
                            nc.tensor.matmul(
                                ps[:], wb1s_g[gi][:, ki, m * 128:(m + 1) * 128],
                                d["bt"][gi][:, ki, :],
                                start=(k == 0), stop=(k == NKF - 1))
                        nc.scalar.activation(d["act1"][:, m, :], ps[:], AF.Relu,
                                             bias=bb1s[:, m:m + 1])
                    return f

                def th_l2(r, m):
                    def f():
                        d = blocks[r]
                        ps = psA.tile([128, BLK], F32, tag="ps", name=f"psl2_{r}_{m}")
                        for k in range(8):
                            nc.tensor.matmul(
                                ps[:], wb2s[:, k, m * 128:(m + 1) * 128],
                                d["act1"][:, k, :], start=(k == 0), stop=(k == 7))
                        nc.scalar.activation(d["act2"][:, m, :], ps[:], AF.Relu,
                                             bias=bb2s[:, m:m + 1])
                    return f

                def th_l3(r, m):
                    def f():
                        d = blocks[r]
                        ps = psA.tile([128, BLK], F32, tag="ps", name=f"psl3_{r}_{m}")
                        for k in range(2):
                            nc.tensor.matmul(
                                ps[:], wb3s[:, k, m * 128:(m + 1) * 128],
                                d["act2"][:, k, :], start=(k == 0), stop=(k == 1))
                        nc.scalar.activation(d["xt"][:, m, :], ps[:], AF.Relu,
                                             bias=bb3s[:, m:m + 1])
                    return f

                def th_ord(r):
                    def f():
                        d = blocks[r]
                        ps = psA.tile([128, BLK], F32, tag="ps", name=f"pso1_{r}")
                        nc.tensor.matmul(ps[:], wo1s[:], d["ot"][:],
                                         start=True, stop=True)
                        o1 = apool.tile([128, BLK], BF, tag="o1", name=f"o1_{r}")
                        nc.scalar.activation(o1[:], ps[:], AF.Relu,
                                             bias=bo1s[:, 0:1])
                        ps = psA.tile([128, BLK], F32, tag="ps", name=f"pso2_{r}")
                        nc.tensor.matmul(ps[:], wo2s[:], o1[:],
                                         start=True, stop=True)
                        nc.scalar.activation(d["xt"][:, 2, :], ps[:], AF.Relu,
                                             bias=bo2s[:, 0:1])
                    return f

                def th_msg(r):
                    def f():
                        d = blocks[r]
                        ps = psA.tile([128, BLK], F32, tag="ps", name=f"psm1_{r}")
                        nc.tensor.matmul(ps[:64, :], wm1s[:], d["mt"][:],
                                         start=True, stop=True)
                        m1 = apool.tile([64, BLK], BF, tag="m1", name=f"m1_{r}")
                        nc.scalar.activation(m1[:], ps[:64, :], AF.Relu,
                                             bias=bm1s[:, 0:1])
                        ps = psA.tile([128, BLK], F32, tag="ps", name=f"psm2_{r}")
                        nc.tensor.matmul(ps[:], wm2s[:], m1[:],
                                         start=True, stop=True)
                        nc.scalar.activation(d["xt"][:, 3, :], ps[:], AF.Relu,
                                             bias=bm2s[:, 0:1])
                    return f

                def th_xw(r, m):
                    def f():
                        d = blocks[r]
                        if m == 0:
                            d["xw"] = xwpool.tile([128, 16, BLK], BF, tag="xw",
                                                  name=f"xw{r}")
                        ps = psA.tile([128, BLK], F32, tag="ps", name=f"psxw_{r}_{m}")
                        for k in range(4):
                            nc.tensor.matmul(
                                ps[:], wihs[:, k, m * 128:(m + 1) * 128],
                                d["xt"][:, k, :], start=(k == 0), stop=(k == 3))
                        nc.scalar.activation(d["xw"][:, m, :], ps[:], AF.Identity,
                                             bias=bgs[:, m:m + 1])
                    return f

                work = []       # (cost_ns, thunk, block)
                done_upto = {}  # block -> index in work after which it's done
                for r in range(NBLK):
                    work.append((400, th_dma(r), r))
                    for m in range(8):
                        work.append((3400, th_l1(r, m), r))
                    for m in range(2):
                        work.append((1300, th_l2(r, m), r))
                    for m in range(2):
                        work.append((500, th_l3(r, m), r))
                    work.append((700, th_ord(r), r))
                    work.append((700, th_msg(r), r))
                    for m in range(16):
                        work.append((900, th_xw(r, m), r))
                    done_upto[r] = len(work)
                widx = 0

                def drain_until_block(r):
                    nonlocal widx
                    tgt = done_upto[min(r, NBLK - 1)]
                    while widx < tgt:
                        work[widx][1]()
                        widx += 1

                def drain_budget(ns):
                    nonlocal widx
                    spent = 0
                    while widx < len(work) and spent < ns:
                        spent += work[widx][0]
                        work[widx][1]()
                        widx += 1

                # prime: blocks 0 and 1 fully emitted before the LSTM starts
                drain_until_block(1)

                XBLK = 8  # steps per xw prefetch region (320 xw columns)
                RC = XBLK * LROWS
                for t in range(S):
                    r = t // XBLK
                    if t % XBLK == 0:
                        # region r == branch block r: consume its xW tile
                        # straight out of SBUF, no DRAM round trip
                        drain_until_block(r)
                        xwin = blocks[r]["xw"]
                    ts0 = (t % XBLK) * LROWS
                    # Gate order g,i,f,o: everything except sig(o) and the
                    # final h-mul completes while the matmul stream is still
                    # running, so the serial tail is just sig(o) -> h.
                    gorder = (2, 0, 1, 3)
                    pss = {}
                    ga = {}
                    fc = lpool.tile([128, 4 * LROWS], F32, tag="fc",
                                    name=f"fc{t}")
                    for g in gorder:
                        ps = psB.tile([128, 4 * LROWS], F32, tag="gps",
                                      name=f"ps{g}_{t}")
                        pss[g] = ps
                        # seed gate PSUM with the xW term (identity matmul,
                        # no hh dependency), then accumulate W_hh matmuls
                        # kk-major so the stream starts on hh[0] alone.
                        nc.tensor.matmul(
                            ps[:], ids[:],
                            xwin[:, 4 * g:4 * g + 4, ts0:ts0 + LROWS],
                            start=True, stop=False, skip_group_check=True)
                        for kk in range(4):
                            for k in range(4):
                                nc.tensor.matmul(
                                    ps[:, k * LROWS:(k + 1) * LROWS],
                                    whhs[:, kk, (4 * g + k) * 128:(4 * g + k + 1) * 128],
                                    hh[kk][:], start=False,
                                    stop=(kk == 3 and k == 3),
                                    skip_group_check=True)
                        if g == 3:
                            break
                        gact = lpool.tile([128, 4 * LROWS], F32, tag=f"ga{g}",
                                          name=f"ga{g}_{t}")
                        fn = AF.Tanh if g == 2 else AF.Sigmoid
                        nc.scalar.activation(gact[:], ps[:], fn)
                        ga[g] = gact
                        if g == 1:
                            for k in range(4):
                                nc.gpsimd.tensor_mul(
                                    fc[:, k * LROWS:(k + 1) * LROWS],
                                    gact[:, k * LROWS:(k + 1) * LROWS], cc[k][:])
                            # cell chains: run under the o-gate matmuls
                            tcs = []
                            for k in range(4):
                                ks = slice(k * LROWS, (k + 1) * LROWS)
                                ig = lpool.tile([128, LROWS], F32, tag=f"ig{k}",
                                                name=f"ig{k}_{t}")
                                nc.vector.tensor_mul(ig[:], ga[0][:, ks],
                                                     ga[2][:, ks])
                                nc.vector.tensor_add(cc[k][:], ig[:], fc[:, ks])
                                tcell = lpool.tile([128, LROWS], F32,
                                                   tag=f"tc{k}", name=f"tc{k}_{t}")
                                nc.scalar.activation(tcell[:], cc[k][:], AF.Tanh)
                                tcs.append(tcell)
                    # tail: sig(o) -> h (chunk 0 in its own tile so h0
                    # doesn't wait on the remaining chunks' sigmoid)
                    so0 = lpool.tile([128, LROWS], F32, tag="so0",
                                     name=f"so0_{t}")
                    nc.scalar.activation(so0[:], pss[3][:, :LROWS], AF.Sigmoid)
                    sor = lpool.tile([128, 3 * LROWS], F32, tag="sor",
                                     name=f"sor{t}")
                    nc.vector.tensor_mul(hh[0][:], so0[:], tcs[0][:])
                    nc.scalar.activation(sor[:], pss[3][:, LROWS:], AF.Sigmoid)
                    for k in range(1, 4):
                        nc.vector.tensor_mul(hh[k][:],
                                             sor[:, (k - 1) * LROWS:k * LROWS],
                                             tcs[k][:])
                    for k in range(4):
                        nc.vector.tensor_add(qq[k][:], qq[k][:], hh[k][:])
                    # fill the recurrence latency with branch work
                    drain_budget(4000)
                drain_budget(1 << 30)

                # ===================== head =====================
                qb = lpool.tile([128, 4, LROWS], BF, tag="qb")
                for k in range(4):
                    nc.vector.tensor_copy(qb[:, k, :], qq[k][:])
                e1 = []
                for m in range(2):
                    ps = psB.tile([128, LROWS], F32, tag="gps")
                    for k in range(4):
                        nc.tensor.matmul(
                            ps[:], w1s[:, k, m * 128:(m + 1) * 128],
                            qb[:, k, :], start=(k == 0), stop=False)
                    nc.tensor.matmul(
                        ps[:], w1s[:2 * A, 4, m * 128:(m + 1) * 128],
                        oms[:], start=False, stop=True)
                    e1m = lpool.tile([128, LROWS], BF, tag=f"e1{m}")
                    nc.scalar.activation(e1m[:], ps[:], AF.Relu, bias=b1s[:, m:m + 1])
                    e1.append(e1m)
                for m in range(2):
                    ps = psB.tile([128, LROWS], F32, tag="gps")
                    for k in range(2):
                        nc.tensor.matmul(
                            ps[:], w2s[:, k, m * 128:(m + 1) * 128],
                            e1[k][:], start=(k == 0), stop=(k == 1))
                    eo = lpool.tile([128, LROWS], F32, tag=f"eo{m}")
                    nc.scalar.activation(eo[:], ps[:], AF.Identity, bias=b2s[:, m:m + 1])
                    nc.sync.dma_start(out=echar[m * 128:(m + 1) * 128, :], in_=eo[:])

    _split_multi_waits(nc)
    return nc


_NC_CACHE = {}


def _get_nc():
    if "nc" not in _NC_CACHE:
        _NC_CACHE["nc"] = _build()
    return _NC_CACHE["nc"]


def _prep(inputs):
    f32 = np.float32
    board = np.asarray(inputs["board"], dtype=f32).reshape(B, P, S, FEAT)
    order = np.asarray(inputs["order"], dtype=f32)
    message = np.asarray(inputs["message"], dtype=f32)
    other = np.asarray(inputs["other_ind"], dtype=f32)
    me = np.asarray(inputs["me_ind"], dtype=f32)
    hx0 = np.asarray(inputs["hx0"], dtype=f32)
    cx0 = np.asarray(inputs["cx0"], dtype=f32)

    w = {}
    wb1t_p = np.zeros((FEATP, 1024), dtype=BF16)
    wb1t_p[:FEAT] = np.ascontiguousarray(inputs["Wb1"].T).astype(BF16)
    w["wb1t"] = wb1t_p
    w["wb2t"] = np.ascontiguousarray(inputs["Wb2"].T).astype(BF16)
    w["wb3t"] = np.ascontiguousarray(inputs["Wb3"].T).astype(BF16)
    w["wo1t"] = np.ascontiguousarray(inputs["Wo1"].T).astype(BF16)
    w["wo2t"] = np.ascontiguousarray(inputs["Wo2"].T).astype(BF16)
    w["wm1t"] = np.ascontiguousarray(inputs["Wm1"].T).astype(BF16)
    w["wm2t"] = np.ascontiguousarray(inputs["Wm2"].T).astype(BF16)
    w["wiht"] = np.ascontiguousarray(inputs["W_ih"].T).astype(BF16)
    w["whht"] = np.ascontiguousarray(inputs["W_hh"].T).astype(BF16)
    w["w1t"] = np.ascontiguousarray(inputs["W1"].T).astype(BF16)
    w["w2t"] = np.ascontiguousarray(inputs["W2"].T).astype(BF16)
    w["bb1"] = np.ascontiguousarray(
        np.asarray(inputs["bb1"], f32).reshape(8, 128).T)
    w["bb2"] = np.ascontiguousarray(
        np.asarray(inputs["bb2"], f32).reshape(2, 128).T)
    w["bb3"] = np.ascontiguousarray(
        np.asarray(inputs["bb3"], f32).reshape(2, 128).T)
    w["bo1"] = np.asarray(inputs["bo1"], f32).reshape(128, 1)
    w["bo2"] = np.asarray(inputs["bo2"], f32).reshape(128, 1)
    w["bm1"] = np.asarray(inputs["bm1"], f32).reshape(64, 1)
    w["bm2"] = np.asarray(inputs["bm2"], f32).reshape(128, 1)
    w["bg"] = np.ascontiguousarray(
        (np.asarray(inputs["b_ih"], f32)
         + np.asarray(inputs["b_hh"], f32)).reshape(16, 128).T)
    w["b1"] = np.ascontiguousarray(
        np.asarray(inputs["b1"], f32).reshape(2, 128).T)
    w["b2"] = np.ascontiguousarray(
        np.asarray(inputs["b2"], f32).reshape(2, 128).T)
    w["ident"] = np.eye(128, dtype=np.float32).astype(BF16)

    board_bf = board.astype(BF16)        # [B,P,S,FEAT]
    order_bf = order.astype(BF16)
    msg_bf = message.astype(BF16)
    hT = hx0.transpose(2, 0, 1)          # [H,P,B]
    cT = cx0.transpose(2, 0, 1)
    oT = other.transpose(2, 1, 0)        # [A,P,B]
    mT = me.transpose(2, 1, 0)

    in_maps = []
    for c in range(NCORES):
        bsl = slice(c * BC, (c + 1) * BC)
        im = dict(w)
        bT = np.zeros((FEATP, ROWS), dtype=BF16)
        bT[:FEAT] = np.ascontiguousarray(
            board_bf[bsl].transpose(3, 2, 1, 0)).reshape(FEAT, ROWS)
        im["boardT"] = bT
        im["orderT"] = np.ascontiguousarray(
            order_bf[bsl].transpose(3, 2, 1, 0)).reshape(OF, ROWS)
        im["msgT"] = np.ascontiguousarray(
            msg_bf[bsl].transpose(3, 2, 1, 0)).reshape(MF, ROWS)
        im["h0"] = np.ascontiguousarray(hT[:, :, bsl]).reshape(H, LROWS).astype(BF16)
        im["c0"] = np.ascontiguousarray(cT[:, :, bsl]).reshape(H, LROWS)
        im["omt"] = np.concatenate(
            [np.ascontiguousarray(oT[:, :, bsl]).reshape(A, LROWS),
             np.ascontiguousarray(mT[:, :, bsl]).reshape(A, LROWS)],
            axis=0).astype(BF16)
        in_maps.append(im)
    return in_maps


_RUN_OPTS = {"trace": False}
_LAST_RES = {}


def kernel(**inputs):
    nc = _get_nc()
    in_maps = _prep(inputs)
    res = run_bass_kernel_spmd(nc, in_maps, list(range(NCORES)),
                               trace=_RUN_OPTS["trace"])
    _LAST_RES["res"] = res
    ect = np.empty((P, B, C), dtype=np.float32)
    for c in range(NCORES):
        out_c = res.results[c]["echar"]              # [C, LROWS]
        ect[:, c * BC:(c + 1) * BC, :] = (
            out_c.reshape(C, P, BC).transpose(1, 2, 0))
    final = ect.sum(axis=0)
    return final, ect
